# revision 24
# baseline (speedup 1.0000x reference)
"""GAT (2-layer, 4-head, segment-softmax) message-passing kernel for 8 Trainium2
NeuronCores.

Strategy (dst-sharded, edge aggregation as one-hot matmuls, factored softmax):
  * Nodes are degree-strided across 784 groups of 128 slots (8 cores x 98
    groups) so every group carries a near-equal edge load; the permutation is
    (core, group, slot) order.
  * Per layer, each core computes records only for its OWN nodes
    (rec[n_own] = [xh(256) | u=exp(a_s)(4) | p=exp(0.2 a_s)(4) | pad], bf16)
    plus a dst-side table vq[n_own] = [v=exp(a_d) | q=exp(0.2 a_d)]; an
    8-core AllGather assembles the full record table.
  * exp(lrelu(a_s+a_d)) == max(u*v, p*q) exactly (exp is monotone), so the
    per-edge attention numerator needs only elementwise ops on gathered
    values - no per-chunk transpose/matmul broadcast.
  * For each destination group, the core gathers the in-edges' source
    records with gpsimd dma_gather (int16 indices, 32768-row buckets) and
    the dst-side vq rows (single bucket, local), builds the one-hot
    incidence M[edge, dst_slot] on the vector engine, and reduces both the
    softmax denominators and weighted feature sums with PSUM-accumulated
    matmuls contracting over edges. Softmax normalization is applied on the
    dst side after the reduction (the max-subtraction of the reference is a
    denominator-cancelling no-op at these magnitudes).
  * Head-mean + LayerNorm + ReLU + residual run per group on vector/scalar
    engines; h stays resident in SBUF between layers; the final projection
    is fused into layer 2's group loop.
"""

import sys

sys.path.insert(0, "/opt/trn_rl_repo")

import numpy as np

# ---- problem constants (hardcoded; kernel.py must be self-contained) ----
N = 100000
E = 1600000
G = 64
H = 4
CDIM = 64
NODE_F = 32
DRONE_F = 16
OUT_F = 32
LN_EPS = 1e-5
NCORES = 8
P = 128
HC = H * CDIM          # 256
REC = HC + 2 * H       # 264: [xh(256) | u(4) | p(4)]
RECP = 384             # padded record elems (bf16 row = 768B, mult of 256)
VQW = 128              # vq table row elems (bf16 row = 256B)
NGROUP = 98
NPC = NGROUP * P       # 12544 padded rows per core
NPAD = NCORES * NPC    # 100352
BUCKET = 32768
NBUCKETS = -(-NPAD // BUCKET)  # 4
TB = 7                 # phase-1 tile batch (98 = 14*7)


class _Cfg:
    def __init__(self, chg, callbase):
        self.chg = chg                # chunks per group (shared across cores)
        self.callbase = callbase      # [NGROUP][ncalls] gather-window bases
        self.chmax = max(chg)
        self.cols = sum(chg)


# --------------------------------------------------------------------------
# host-side preprocessing
# --------------------------------------------------------------------------

def _host_prep(edge_index):
    """Degree-strided node permutation + per-core int16 gather streams."""
    loop = np.arange(N, dtype=np.int64)
    src = np.concatenate([edge_index[0].astype(np.int64), loop])
    dst = np.concatenate([edge_index[1].astype(np.int64), loop])
    deg = np.bincount(dst, minlength=N)

    # node rank i (by degree desc) -> grp784 = i % 784, slot = i // 784
    order_by_deg = np.argsort(-deg, kind="stable")
    NG = NCORES * NGROUP
    grp784 = np.arange(N) % NG
    slot784 = np.arange(N) // NG
    core_of_g = grp784 % NCORES
    group_of_g = grp784 // NCORES
    pos_of = np.empty(N, np.int64)            # node -> padded global position
    pos_of[order_by_deg] = core_of_g * NPC + group_of_g * P + slot784
    order = np.full(NPAD, -1, np.int64)       # padded position -> node
    order[pos_of] = np.arange(N)

    e_core = pos_of[dst] // NPC
    e_group = (pos_of[dst] % NPC) // P
    e_slot = pos_of[dst] % P

    # per-core edge streams sorted by (group, src position); chunk schedule
    # shared across cores: chg[g] = max_k ceil(E_gk / 128). Gather windows
    # are per 256-idx call (2 chunks), base = min over cores of the call's
    # first src position (sliding window; no bucket-grid round-up).
    streams = []
    cnt_kg = np.zeros((NCORES, NGROUP), np.int64)
    for k in range(NCORES):
        mask = e_core == k
        es = pos_of[src[mask]]
        eg = e_group[mask]
        esl = e_slot[mask]
        o = np.lexsort((es, eg))
        es, eg, esl = es[o], eg[o], esl[o]
        np.add.at(cnt_kg[k], eg, 1)
        streams.append((es, eg, esl))
    chg_np = -(-cnt_kg.max(axis=0) // P)                   # [NGROUP]
    chg = [int(c) for c in chg_np]
    goff = np.concatenate([[0], np.cumsum(chg_np)])[:-1]
    ncalls = -(-chg_np // 2)
    cumcalls = np.concatenate([[0], np.cumsum(ncalls)])
    tot_calls = int(cumcalls[-1])

    # call bases: min over cores of first src pos in each call's edge range
    first = np.full((NCORES, tot_calls), np.iinfo(np.int64).max, np.int64)
    last = np.zeros((NCORES, tot_calls), np.int64)
    for k in range(NCORES):
        es, eg, esl = streams[k]
        gstart = np.concatenate([[0], np.cumsum(cnt_kg[k])])[:-1]
        j = np.arange(len(es)) - gstart[eg]
        call = cumcalls[eg] + j // 256
        np.minimum.at(first[k], call, es)
        np.maximum.at(last[k], call, es)
    callbase_flat = first.min(axis=0)
    empty = callbase_flat == np.iinfo(np.int64).max
    callbase_flat[empty] = 0
    span = last.max(axis=0) - callbase_flat
    assert span.max() < BUCKET, f"gather window overflow: {span.max()}"
    callbase = [[int(callbase_flat[cumcalls[g] + i]) for i in range(ncalls[g])]
                for g in range(NGROUP)]
    cfg = _Cfg(chg, callbase)
    cols = cfg.cols

    per_core = []
    for k in range(NCORES):
        es, eg, esl = streams[k]
        gstart = np.concatenate([[0], np.cumsum(cnt_kg[k])])[:-1]
        j = np.arange(len(es)) - gstart[eg]
        slotj = goff[eg] * P + j              # global edge slot in stream
        call = cumcalls[eg] + j // 256
        rel = es - callbase_flat[call]

        idx16 = np.zeros((16, cols * 8), np.int16)
        idx16[slotj % 16, slotj // 16] = rel
        # pad slots point at the zeroed row NPC of the vq table and carry
        # dst slot -1, which never one-hot matches
        vqidx16 = np.full((16, cols * 8), NPC, np.int16)
        vqidx16[slotj % 16, slotj // 16] = eg * P + esl
        dstslot8 = np.full((P, cols), -1, np.int8)
        dstslot8[slotj % P, slotj // P] = esl
        per_core.append(dict(idx16=idx16, vqidx16=vqidx16, dstslot8=dstslot8))
    return dict(order=order, pos_of=pos_of, cfg=cfg, per_core=per_core)


def _host_weights(inputs, order):
    """Permuted inputs + combined weights (f32/bf16)."""
    import ml_dtypes
    f = np.float32
    bf = ml_dtypes.bfloat16
    valid = order >= 0
    xp = np.zeros((NPAD, NODE_F), f)
    xp[valid] = np.asarray(inputs["x"], f)[order[valid]]
    batchp = np.zeros(NPAD, np.int32)
    batchp[valid] = np.asarray(inputs["batch"], np.int32)[order[valid]]

    dr64 = (np.asarray(inputs["drone_feat"], f) @ np.asarray(inputs["drone_W"], f).T
            + np.asarray(inputs["drone_b"], f) + np.asarray(inputs["node_b"], f))

    out = dict(nodeWT=np.ascontiguousarray(
                   np.asarray(inputs["node_W"], f).T.astype(bf)),  # [NODE_F, C]
               dr64=dr64.astype(f),
               outWT=np.ascontiguousarray(np.asarray(inputs["out_W"], f).T),
               outb=np.asarray(inputs["out_b"], f).reshape(1, OUT_F))
    for l in range(2):
        W = np.asarray(inputs[f"convW{l}"], f)       # [HC, CDIM]
        a_s = np.asarray(inputs[f"att_src{l}"], f)   # [H, CDIM]
        a_d = np.asarray(inputs[f"att_dst{l}"], f)
        Wh = W.reshape(H, CDIM, CDIM)
        Ws = np.einsum("hcf,hc->fh", Wh, a_s)        # [CDIM, H]
        Wd = np.einsum("hcf,hc->fh", Wh, a_d)
        out[f"wcomb{l}"] = np.concatenate([W.T, Ws, Wd], 1)   # [CDIM, 264]
        out[f"convb{l}"] = np.asarray(inputs[f"convb{l}"], f).reshape(1, CDIM)
        out[f"lng{l}"] = np.asarray(inputs[f"ln_g{l}"], f).reshape(1, CDIM)
        out[f"lnb{l}"] = np.asarray(inputs[f"ln_b{l}"], f).reshape(1, CDIM)

    per_core = []
    for k in range(NCORES):
        sl = slice(k * NPC, (k + 1) * NPC)
        per_core.append(dict(
            xTa=np.ascontiguousarray(xp[sl].T.astype(bf)),        # [32, NPC]
            batchc=np.ascontiguousarray(
                batchp[sl].reshape(NGROUP, P).T.astype(np.int32)),  # [P, 98]
        ))
    return out, per_core


# --------------------------------------------------------------------------
# numpy emulation of the device schedule (for validation, no HW)
# --------------------------------------------------------------------------

def _emulate(cfg, prep, shared, per_core_w):
    import ml_dtypes
    bf = ml_dtypes.bfloat16
    f = np.float32
    rec = np.zeros((NPAD, RECP), bf)
    vq = np.zeros((NCORES, NPC + 1, 8), bf)
    had = np.zeros((NCORES, NPC, CDIM), f)
    out = np.zeros((NCORES, NPC, OUT_F), np.float16)

    # phase1 layer 0: h0 = x @ nodeW.T + dr64[batch]
    for k in range(NCORES):
        xT = per_core_w[k]["xTa"].astype(f)       # [32, NPC]
        bc = per_core_w[k]["batchc"]              # [P, 98]
        h0 = xT.T @ shared["nodeWT"].astype(f) + shared["dr64"][
            bc.T.reshape(-1)]
        had[k] = h0

    def phase1(l):
        for k in range(NCORES):
            prc = had[k] @ shared[f"wcomb{l}"]    # [NPC, 264]
            sl = slice(k * NPC, (k + 1) * NPC)
            rec[sl, 0:HC] = prc[:, 0:HC].astype(bf)
            rec[sl, HC:HC + H] = np.exp(prc[:, HC:HC + H]).astype(bf)
            rec[sl, HC + H:REC] = np.exp(0.2 * prc[:, HC:HC + H]).astype(bf)
            vq[k, :NPC, 0:4] = np.exp(prc[:, HC + H:REC]).astype(bf)
            vq[k, :NPC, 4:8] = np.exp(0.2 * prc[:, HC + H:REC]).astype(bf)

    def phase2(l):
        for k in range(NCORES):
            pc = prep["per_core"][k]
            col0 = 0
            for g in range(NGROUP):
                CH = cfg.chg[g]
                s = np.arange(CH * P)
                idx = pc["idx16"][(col0 * P + s) % 16,
                                  (col0 * P + s) // 16].astype(np.int64)
                # add per-call window bases
                cb_arr = np.asarray(cfg.callbase[g], np.int64)
                idx += cb_arr[s // 256]
                vqi = pc["vqidx16"][(col0 * P + s) % 16,
                                    (col0 * P + s) // 16].astype(np.int64)
                ds = pc["dstslot8"][:, col0:col0 + CH].astype(np.int64)
                rect = rec[idx].reshape(CH, P, RECP).transpose(1, 0, 2)
                vqt = vq[k][vqi].reshape(CH, P, 8).transpose(1, 0, 2)
                t1 = rect[:, :, HC:HC + H].astype(f) * vqt[:, :, 0:4].astype(f)
                t2 = rect[:, :, HC + H:REC].astype(f) * vqt[:, :, 4:8].astype(f)
                ex = np.maximum(t1, t2).astype(bf).astype(f)  # [P, CH, 4]
                xh = rect[:, :, 0:HC].astype(f).reshape(P, CH, H, CDIM)
                v = (xh * ex[:, :, :, None]).astype(bf).astype(f)
                M = (ds[:, :, None] == np.arange(P)[None, None, :])  # [P,CH,P]
                Mb = M.astype(bf).astype(f)
                # contract edges: pg[d, 0:256] / s4[d, 4]
                vflat = v.reshape(P, CH, HC)
                pg = np.einsum("ecd,ecf->df", Mb, vflat)
                s4 = np.einsum("ecd,ech->dh", Mb, ex)
                r4 = 1.0 / (s4 + 1e-16) / H
                yt = np.zeros((P, CDIM), f)
                for h_ in range(H):
                    yt += pg[:, h_ * CDIM:(h_ + 1) * CDIM] * r4[:, h_:h_ + 1]
                yt += shared[f"convb{l}"]
                mu = yt.mean(1, keepdims=True)
                var = ((yt - mu) ** 2).mean(1, keepdims=True)
                yt = (yt - mu) / np.sqrt(var + LN_EPS)
                yt = yt * shared[f"lng{l}"] + shared[f"lnb{l}"]
                yt = np.maximum(yt, 0.0)
                rows = slice(g * P, (g + 1) * P)
                yt2 = yt + had[k][rows]
                if l == 0:
                    had[k][rows] = yt2
                else:
                    out[k, rows] = (yt2 @ shared["outWT"]
                                    + shared["outb"]).astype(np.float16)
                col0 += CH

    phase1(0); phase2(0); phase1(1); phase2(1)
    return out


# --------------------------------------------------------------------------
# bass kernel
# --------------------------------------------------------------------------

def _build(cfg):
    import concourse.bass as bass
    import concourse.bacc as bacc
    import concourse.tile as tile
    from concourse import mybir
    from concourse.masks import make_identity

    f32 = mybir.dt.float32
    f16 = mybir.dt.float16
    i32 = mybir.dt.int32
    i16 = mybir.dt.int16
    bf16 = mybir.dt.bfloat16
    Alu = mybir.AluOpType
    Act = mybir.ActivationFunctionType

    CHMAX, cols = cfg.chmax, cfg.cols

    nc = bacc.Bacc("TRN2", target_bir_lowering=False, debug=False,
                   num_devices=NCORES)

    def ein(nm, sh, dt=f32):
        return nc.dram_tensor(nm, sh, dt, kind="ExternalInput")

    xTa_d = ein("xTa", [NODE_F, NPC], bf16)
    batchc_d = ein("batchc", [P, NGROUP], i32)
    dr64_d = ein("dr64", [G, CDIM])
    nodeWT_d = ein("nodeWT", [NODE_F, CDIM], bf16)
    wcomb_d = [ein(f"wcomb{l}", [CDIM, REC]) for l in range(2)]
    convb_d = [ein(f"convb{l}", [1, CDIM]) for l in range(2)]
    lng_d = [ein(f"lng{l}", [1, CDIM]) for l in range(2)]
    lnb_d = [ein(f"lnb{l}", [1, CDIM]) for l in range(2)]
    outWT_d = ein("outWT", [CDIM, OUT_F])
    outb_d = ein("outb", [1, OUT_F])
    idx16_d = ein("idx16", [16, cols * 8], i16)
    vqidx16_d = ein("vqidx16", [16, cols * 8], i16)
    dstslot8_d = ein("dstslot8", [P, cols], mybir.dt.int8)

    out_d = nc.dram_tensor("out", [NPC, OUT_F], f16, kind="ExternalOutput")

    rec_own_d = nc.dram_tensor("rec_own", [NPC, RECP], bf16)
    rec_d = nc.dram_tensor("rec", [NPAD, RECP], bf16, addr_space="Shared")
    vq_d = nc.dram_tensor("vq", [NPC + 1, VQW], bf16)
    idx128_d = nc.dram_tensor("idx128", [P, cols * 8], i16)
    vqidx128_d = nc.dram_tensor("vqidx128", [P, cols * 8], i16)

    from contextlib import ExitStack
    with tile.TileContext(nc) as tc, ExitStack() as ctx:
        cpool = ctx.enter_context(tc.tile_pool(name="const", bufs=1))
        p1 = ctx.enter_context(tc.tile_pool(name="p1", bufs=2))
        p2 = ctx.enter_context(tc.tile_pool(name="p2", bufs=2))

        def cload(dram):
            t = cpool.tile(list(dram.shape), dram.dtype, tag=f"c_{dram.name}")
            nc.sync.dma_start(out=t[:], in_=dram[:])
            return t

        xTa_sb = cload(xTa_d)
        batchc_sb = cload(batchc_d)
        nodeWT_sb = cload(nodeWT_d)
        wcomb_sb = [cload(d) for d in wcomb_d]
        outWT_sb = cload(outWT_d)

        ds8_sb = cload(dstslot8_d)

        # replicate [16, X] index streams to [128, X] in DRAM
        for k in range(8):
            nc.sync.dma_start(out=idx128_d[k * 16:(k + 1) * 16, :],
                              in_=idx16_d[:, :])
            nc.sync.dma_start(out=vqidx128_d[k * 16:(k + 1) * 16, :],
                              in_=vqidx16_d[:, :])

        iota_sb = cpool.tile([P, P], i32)
        nc.gpsimd.iota(iota_sb[:], pattern=[[1, P]], base=0,
                       channel_multiplier=0)
        iota8_sb = cpool.tile([P, P], mybir.dt.int8)
        nc.vector.tensor_copy(iota8_sb[:], iota_sb[:])
        ident_sb = cpool.tile([P, P], f32)
        make_identity(nc, ident_sb[:])

        # broadcast [1, C] channel weights to all partitions via outer
        # product with a ones column
        ones_sb = cpool.tile([1, P], f32)
        nc.vector.memset(ones_sb[:], 1.0)

        def crep(dram, width):
            row = cpool.tile([1, width], f32, tag=f"r_{dram.name}")
            nc.sync.dma_start(out=row[:], in_=dram[:])
            t = cpool.tile([P, width], f32, tag=f"b_{dram.name}")
            with tc.tile_pool(name=f"pb_{dram.name}", bufs=1,
                              space="PSUM") as pb:
                pt = pb.tile([P, width], f32)
                nc.tensor.matmul(pt[:], lhsT=ones_sb[:], rhs=row[:],
                                 start=True, stop=True)
                nc.scalar.copy(t[:], pt[:])
            return t

        convb_sb = [crep(d, CDIM) for d in convb_d]
        lng_sb = [crep(d, CDIM) for d in lng_d]
        lnb_sb = [crep(d, CDIM) for d in lnb_d]
        outb_sb = crep(outb_d, OUT_F)

        # zero row NPC of the vq table (pad-slot target)
        zrow_sb = cpool.tile([1, VQW], bf16)
        nc.vector.memset(zrow_sb[:], 0.0)
        nc.sync.dma_start(out=vq_d[NPC:NPC + 1, :], in_=zrow_sb[:])

        # h rows (residual / layer-1 input), SBUF-resident
        had_sb = cpool.tile([P, NGROUP, CDIM], f32, tag="had")

        # ------------------------------------------------------------------
        def phase1(l):
            with tc.tile_pool(name=f"ps1_{l}", bufs=2, space="PSUM") as pp:
                for b0 in range(0, NGROUP, TB):
                    recb = p1.tile([P, TB, RECP], bf16, tag="recb")
                    vqb = p1.tile([P, TB, 8], bf16, tag="vqb")
                    for tt in range(TB):
                        t = b0 + tt
                        if l == 0:
                            ph = pp.tile([P, CDIM], f32, tag="ph")
                            nc.tensor.matmul(ph[:],
                                             lhsT=xTa_sb[:, t * P:(t + 1) * P],
                                             rhs=nodeWT_sb[:], start=True,
                                             stop=True)
                            drt = p1.tile([P, CDIM], f32, tag="drt")
                            nc.gpsimd.indirect_dma_start(
                                out=drt[:], out_offset=None, in_=dr64_d[:],
                                in_offset=bass.IndirectOffsetOnAxis(
                                    ap=batchc_sb[:, t:t + 1], axis=0))
                            nc.vector.tensor_add(had_sb[:, t, :], ph[:],
                                                 drt[:])
                        pt = pp.tile([CDIM, P], f32, tag="pt")
                        nc.tensor.transpose(pt[:], had_sb[:, t, :],
                                            ident_sb[:])
                        hT = p1.tile([CDIM, P], f32, tag="hT")
                        nc.scalar.copy(hT[:], pt[:])
                        prc = pp.tile([P, REC], f32, tag="prc")
                        nc.tensor.matmul(prc[:], lhsT=hT[:],
                                         rhs=wcomb_sb[l][:], start=True,
                                         stop=True)
                        nc.scalar.copy(recb[:, tt, 0:HC], prc[:, 0:HC])
                        nc.scalar.activation(recb[:, tt, HC:HC + H],
                                             prc[:, HC:HC + H], Act.Exp)
                        nc.scalar.activation(recb[:, tt, HC + H:REC],
                                             prc[:, HC:HC + H], Act.Exp,
                                             scale=0.2)
                        nc.scalar.activation(vqb[:, tt, 0:4],
                                             prc[:, HC + H:REC], Act.Exp)
                        nc.scalar.activation(vqb[:, tt, 4:8],
                                             prc[:, HC + H:REC], Act.Exp,
                                             scale=0.2)
                    r0 = b0 * P
                    rows = TB * P
                    nc.sync.dma_start(
                        out=rec_own_d[r0:r0 + rows, :].rearrange(
                            "(c p) f -> p c f", p=P),
                        in_=recb[:, :, :])
                    nc.sync.dma_start(
                        out=vq_d[r0:r0 + rows, 0:8].rearrange(
                            "(c p) f -> p c f", p=P),
                        in_=vqb[:, :, :])
            nc.gpsimd.collective_compute(
                "AllGather", mybir.AluOpType.bypass,
                replica_groups=[list(range(NCORES))],
                ins=[rec_own_d[:, :].opt()],
                outs=[rec_d[:, :].opt()])

        # ------------------------------------------------------------------
        def phase2(l):
            with tc.tile_pool(name=f"ps2_{l}", bufs=2, space="PSUM") as pp:
                col0 = 0
                for g in range(NGROUP):
                    CH = cfg.chg[g]
                    idxt = p2.tile([P, CHMAX * 8], i16, tag="idxt")
                    nc.sync.dma_start(out=idxt[:, :CH * 8],
                                      in_=idx128_d[:, col0 * 8:(col0 + CH) * 8])
                    vqit = p2.tile([P, CHMAX * 8], i16, tag="vqit")
                    nc.sync.dma_start(
                        out=vqit[:, :CH * 8],
                        in_=vqidx128_d[:, col0 * 8:(col0 + CH) * 8])
                    rect = p2.tile([P, CHMAX, RECP], bf16, tag="rect")
                    done = 0
                    i = 0
                    while done < CH:   # HW envelope: <=256 idxs per call
                        st = min(2, CH - done)
                        base = cfg.callbase[g][i]
                        nrows = min(BUCKET, NPAD - base)
                        nc.gpsimd.dma_gather(
                            rect[:, done:done + st, :],
                            rec_d[base:base + nrows, :],
                            idxt[:, done * 8:(done + st) * 8],
                            st * P, st * P, RECP)
                        done += st
                        i += 1
                    vqt = p2.tile([P, CHMAX, VQW], bf16, tag="vqt")
                    done = 0
                    while done < CH:
                        st = min(2, CH - done)
                        nc.gpsimd.dma_gather(
                            vqt[:, done:done + st, :], vq_d[0:NPC + 1, :],
                            vqit[:, done * 8:(done + st) * 8],
                            st * P, st * P, VQW)
                        done += st
                    # one-hot M[edge, dst_slot]
                    Mt = p2.tile([P, CHMAX, P], bf16, tag="Mt")
                    nc.vector.tensor_tensor(
                        Mt[:, :CH, :],
                        ds8_sb[:, col0:col0 + CH][:, :, None].to_broadcast(
                            [P, CH, P]),
                        iota8_sb[:, None, :].to_broadcast([P, CH, P]),
                        Alu.is_equal)
                    # ex = max(u*v, p*q) -> rect[..., 256:260]
                    t1 = p2.tile([P, CHMAX, H], f32, tag="t1")
                    nc.vector.tensor_tensor(t1[:, :CH, :],
                                            rect[:, :CH, HC:HC + H],
                                            vqt[:, :CH, 0:4], Alu.mult)
                    t2 = p2.tile([P, CHMAX, H], f32, tag="t2")
                    nc.vector.tensor_tensor(t2[:, :CH, :],
                                            rect[:, :CH, HC + H:REC],
                                            vqt[:, :CH, 4:8], Alu.mult)
                    nc.vector.tensor_tensor(rect[:, :CH, HC:HC + H],
                                            t1[:, :CH, :], t2[:, :CH, :],
                                            Alu.max)
                    # V = ex * xh (per head, in place)
                    for h_ in range(H):
                        nc.vector.tensor_tensor(
                            rect[:, :CH, h_ * CDIM:(h_ + 1) * CDIM],
                            rect[:, :CH, h_ * CDIM:(h_ + 1) * CDIM],
                            rect[:, :CH, HC + h_:HC + h_ + 1].to_broadcast(
                                [P, CH, CDIM]),
                            Alu.mult)
                    # contract over edges: pg[:, 0:256]=sum ex*xh, [256:260]=s
                    pg = pp.tile([P, HC + H], f32, tag="pg")
                    for c in range(CH):
                        nc.tensor.matmul(pg[:], lhsT=Mt[:, c, :],
                                         rhs=rect[:, c, 0:HC + H],
                                         start=(c == 0), stop=(c == CH - 1))
                    # r = 1 / (s + eps) / H
                    s4 = p2.tile([P, H], f32, tag="s4")
                    nc.vector.tensor_scalar(s4[:], pg[:, HC:HC + H], 1e-16,
                                            None, Alu.add)
                    r4 = p2.tile([P, H], f32, tag="r4")
                    nc.vector.reciprocal(r4[:], s4[:])
                    nc.vector.tensor_scalar_mul(r4[:], r4[:], 1.0 / H)
                    # head mean
                    yt = p2.tile([P, CDIM], f32, tag="yt")
                    tmp = p2.tile([P, CDIM], f32, tag="tmp")
                    nc.vector.tensor_scalar(yt[:], pg[:, 0:CDIM], r4[:, 0:1],
                                            None, Alu.mult)
                    for h_ in range(1, H):
                        nc.vector.tensor_scalar(
                            tmp[:], pg[:, h_ * CDIM:(h_ + 1) * CDIM],
                            r4[:, h_:h_ + 1], None, Alu.mult)
                        nc.vector.tensor_add(yt[:], yt[:], tmp[:])
                    nc.vector.tensor_add(yt[:], yt[:], convb_sb[l][:])
                    # layernorm
                    mu = p2.tile([P, 1], f32, tag="mu")
                    nc.vector.tensor_reduce(mu[:], yt[:], mybir.AxisListType.X,
                                            Alu.add)
                    nc.vector.tensor_scalar_mul(mu[:], mu[:], 1.0 / CDIM)
                    nc.vector.tensor_scalar(yt[:], yt[:], mu[:, 0:1], None,
                                            Alu.subtract)
                    sq = p2.tile([P, CDIM], f32, tag="sq")
                    var = p2.tile([P, 1], f32, tag="var")
                    nc.scalar.activation(sq[:], yt[:], Act.Square,
                                         accum_out=var[:])
                    nc.vector.tensor_scalar(var[:], var[:], 1.0 / CDIM,
                                            LN_EPS, Alu.mult, Alu.add)
                    sd = p2.tile([P, 1], f32, tag="sd")
                    nc.scalar.sqrt(sd[:], var[:])
                    inv = p2.tile([P, 1], f32, tag="inv")
                    nc.vector.reciprocal(inv[:], sd[:])
                    nc.vector.tensor_scalar(yt[:], yt[:], inv[:, 0:1], None,
                                            Alu.mult)
                    nc.vector.tensor_mul(yt[:], yt[:], lng_sb[l][:])
                    nc.vector.tensor_add(yt[:], yt[:], lnb_sb[l][:])
                    nc.vector.tensor_scalar_max(yt[:], yt[:], 0.0)
                    # residual
                    if l == 0:
                        nc.vector.tensor_add(had_sb[:, g, :], yt[:],
                                             had_sb[:, g, :])
                    else:
                        yt2 = p2.tile([P, CDIM], f32, tag="yt2")
                        nc.vector.tensor_add(yt2[:], yt[:], had_sb[:, g, :])
                        pt2 = pp.tile([CDIM, P], f32, tag="pt2")
                        nc.tensor.transpose(pt2[:], yt2[:], ident_sb[:])
                        hT2 = p2.tile([CDIM, P], f32, tag="hT2")
                        nc.scalar.copy(hT2[:], pt2[:])
                        po = pp.tile([P, OUT_F], f32, tag="po")
                        nc.tensor.matmul(po[:], lhsT=hT2[:], rhs=outWT_sb[:],
                                         start=True, stop=True)
                        ot = p2.tile([P, OUT_F], f16, tag="ot")
                        nc.vector.tensor_add(ot[:], po[:], outb_sb[:])
                        nc.sync.dma_start(out=out_d[g * P:(g + 1) * P, :],
                                          in_=ot[:])
                    col0 += CH

        # ------------------------------------------------------------------
        phase1(0)
        phase2(0)
        phase1(1)
        phase2(1)

    nc.compile()
    return nc


# --------------------------------------------------------------------------
# entry point
# --------------------------------------------------------------------------

def _in_maps(cfg, prep, shared, per_core_w):
    shared_m = dict(dr64=shared["dr64"], nodeWT=shared["nodeWT"],
                    outWT=shared["outWT"], outb=shared["outb"])
    # per-core inputs: idx16, vqidx16, xTa, batchc
    for l in range(2):
        for nm in ("wcomb", "convb", "lng", "lnb"):
            shared_m[f"{nm}{l}"] = shared[f"{nm}{l}"]
    maps = []
    for k in range(NCORES):
        m = dict(shared_m)
        m.update(prep["per_core"][k])
        m["xTa"] = per_core_w[k]["xTa"]
        m["batchc"] = per_core_w[k]["batchc"]
        maps.append({k_: np.ascontiguousarray(v) for k_, v in m.items()})
    return maps


def _gather_out(prep, results):
    out = np.empty((N, OUT_F), np.float32)
    order = prep["order"]
    for k in range(NCORES):
        blk = order[k * NPC:(k + 1) * NPC]
        valid = blk >= 0
        out[blk[valid]] = results[k]["out"][valid].astype(np.float32)
    return out


def kernel(**inputs):
    edge_index = np.asarray(inputs["edge_index"])
    prep = _host_prep(edge_index)
    cfg = prep["cfg"]
    shared, per_core_w = _host_weights(inputs, prep["order"])
    nc = _build(cfg)
    maps = _in_maps(cfg, prep, shared, per_core_w)

    from concourse import bass_utils
    res = bass_utils.run_bass_kernel_spmd(nc, maps,
                                          core_ids=list(range(NCORES)))
    return _gather_out(prep, res.results)


# revision 28
# speedup vs baseline: 1.1679x; 1.1679x over previous
"""GAT (2-layer, 4-head, segment-softmax) message-passing kernel for 8 Trainium2
NeuronCores.

Strategy (dst-sharded, edge aggregation as one-hot matmuls, factored softmax):
  * Nodes are degree-strided across 784 groups of 128 slots (8 cores x 98
    groups) so every group carries a near-equal edge load; the permutation is
    (core, group, slot) order.
  * Per layer, each core computes records only for its OWN nodes
    (rec[n_own] = [xh(256) | u=exp(a_s)(4) | p=exp(0.2 a_s)(4) | pad], bf16)
    plus a dst-side table vq[n_own] = [v=exp(a_d) | q=exp(0.2 a_d)]; an
    8-core AllGather assembles the full record table.
  * exp(lrelu(a_s+a_d)) == max(u*v, p*q) exactly (exp is monotone), so the
    per-edge attention numerator needs only elementwise ops on gathered
    values - no per-chunk transpose/matmul broadcast.
  * For each destination group, the core gathers the in-edges' source
    records with gpsimd dma_gather (int16 indices, 32768-row buckets) and
    the dst-side vq rows (single bucket, local), builds the one-hot
    incidence M[edge, dst_slot] on the vector engine, and reduces both the
    softmax denominators and weighted feature sums with PSUM-accumulated
    matmuls contracting over edges. Softmax normalization is applied on the
    dst side after the reduction (the max-subtraction of the reference is a
    denominator-cancelling no-op at these magnitudes).
  * Head-mean + LayerNorm + ReLU + residual run per group on vector/scalar
    engines; h stays resident in SBUF between layers; the final projection
    is fused into layer 2's group loop.
"""

import sys

sys.path.insert(0, "/opt/trn_rl_repo")

import numpy as np

# ---- problem constants (hardcoded; kernel.py must be self-contained) ----
N = 100000
E = 1600000
G = 64
H = 4
CDIM = 64
NODE_F = 32
DRONE_F = 16
OUT_F = 32
LN_EPS = 1e-5
NCORES = 8
P = 128
HC = H * CDIM          # 256
REC = HC + 2 * H       # 264: [xh(256) | u(4) | p(4)]
RECP = 384             # padded record elems (bf16 row = 768B, mult of 256)
VQW = 128              # vq table row elems (bf16 row = 256B)
NGROUP = 98
NPC = NGROUP * P       # 12544 padded rows per core
NPAD = NCORES * NPC    # 100352
BUCKET = 32768
NBUCKETS = -(-NPAD // BUCKET)  # 4
TB = 7                 # phase-1 tile batch (98 = 14*7)


class _Cfg:
    def __init__(self, chg, callbase):
        self.chg = chg                # chunks per group (shared across cores)
        self.callbase = callbase      # [NGROUP][ncalls] gather-window bases
        self.chmax = max(chg)
        self.cols = sum(chg)


# --------------------------------------------------------------------------
# host-side preprocessing
# --------------------------------------------------------------------------

def _host_prep(edge_index):
    """Degree-strided node permutation + per-core int16 gather streams."""
    loop = np.arange(N, dtype=np.int64)
    src = np.concatenate([edge_index[0].astype(np.int64), loop])
    dst = np.concatenate([edge_index[1].astype(np.int64), loop])
    deg = np.bincount(dst, minlength=N)

    # node rank i (by degree desc) -> grp784 = i % 784, slot = i // 784
    order_by_deg = np.argsort(-deg, kind="stable")
    NG = NCORES * NGROUP
    grp784 = np.arange(N) % NG
    slot784 = np.arange(N) // NG
    core_of_g = grp784 % NCORES
    group_of_g = grp784 // NCORES
    pos_of = np.empty(N, np.int64)            # node -> padded global position
    pos_of[order_by_deg] = core_of_g * NPC + group_of_g * P + slot784
    order = np.full(NPAD, -1, np.int64)       # padded position -> node
    order[pos_of] = np.arange(N)

    e_core = pos_of[dst] // NPC
    e_group = (pos_of[dst] % NPC) // P
    e_slot = pos_of[dst] % P

    # per-core edge streams sorted by (group, src position); chunk schedule
    # shared across cores: chg[g] = max_k ceil(E_gk / 128). Gather windows
    # are per 256-idx call (2 chunks), base = min over cores of the call's
    # first src position (sliding window; no bucket-grid round-up).
    streams = []
    cnt_kg = np.zeros((NCORES, NGROUP), np.int64)
    for k in range(NCORES):
        mask = e_core == k
        es = pos_of[src[mask]]
        eg = e_group[mask]
        esl = e_slot[mask]
        o = np.lexsort((es, eg))
        es, eg, esl = es[o], eg[o], esl[o]
        np.add.at(cnt_kg[k], eg, 1)
        streams.append((es, eg, esl))
    chg_np = -(-cnt_kg.max(axis=0) // P)                   # [NGROUP]
    chg = [int(c) for c in chg_np]
    goff = np.concatenate([[0], np.cumsum(chg_np)])[:-1]
    ncalls = -(-chg_np // 2)
    cumcalls = np.concatenate([[0], np.cumsum(ncalls)])
    tot_calls = int(cumcalls[-1])

    # call bases: min over cores of first src pos in each call's edge range
    first = np.full((NCORES, tot_calls), np.iinfo(np.int64).max, np.int64)
    last = np.zeros((NCORES, tot_calls), np.int64)
    for k in range(NCORES):
        es, eg, esl = streams[k]
        gstart = np.concatenate([[0], np.cumsum(cnt_kg[k])])[:-1]
        j = np.arange(len(es)) - gstart[eg]
        call = cumcalls[eg] + j // 256
        np.minimum.at(first[k], call, es)
        np.maximum.at(last[k], call, es)
    callbase_flat = first.min(axis=0)
    empty = callbase_flat == np.iinfo(np.int64).max
    callbase_flat[empty] = 0
    span = last.max(axis=0) - callbase_flat
    assert span.max() < BUCKET, f"gather window overflow: {span.max()}"
    callbase = [[int(callbase_flat[cumcalls[g] + i]) for i in range(ncalls[g])]
                for g in range(NGROUP)]
    cfg = _Cfg(chg, callbase)
    cols = cfg.cols

    per_core = []
    for k in range(NCORES):
        es, eg, esl = streams[k]
        gstart = np.concatenate([[0], np.cumsum(cnt_kg[k])])[:-1]
        j = np.arange(len(es)) - gstart[eg]
        slotj = goff[eg] * P + j              # global edge slot in stream
        call = cumcalls[eg] + j // 256
        rel = es - callbase_flat[call]

        idx16 = np.zeros((16, cols * 8), np.int16)
        idx16[slotj % 16, slotj // 16] = rel
        # pad slots point at the zeroed row NPC of the vq table and carry
        # dst slot -1, which never one-hot matches
        vqidx16 = np.full((16, cols * 8), NPC, np.int16)
        vqidx16[slotj % 16, slotj // 16] = eg * P + esl
        dstslot8 = np.full((P, cols), -1, np.int8)
        dstslot8[slotj % P, slotj // P] = esl
        per_core.append(dict(idx16=idx16, vqidx16=vqidx16, dstslot8=dstslot8))
    return dict(order=order, pos_of=pos_of, cfg=cfg, per_core=per_core)


def _host_weights(inputs, order):
    """Permuted inputs + combined weights (f32/bf16)."""
    import ml_dtypes
    f = np.float32
    bf = ml_dtypes.bfloat16
    valid = order >= 0
    xp = np.zeros((NPAD, NODE_F), f)
    xp[valid] = np.asarray(inputs["x"], f)[order[valid]]
    batchp = np.zeros(NPAD, np.int32)
    batchp[valid] = np.asarray(inputs["batch"], np.int32)[order[valid]]

    dr64 = (np.asarray(inputs["drone_feat"], f) @ np.asarray(inputs["drone_W"], f).T
            + np.asarray(inputs["drone_b"], f) + np.asarray(inputs["node_b"], f))

    out = dict(nodeWT=np.ascontiguousarray(
                   np.asarray(inputs["node_W"], f).T.astype(bf)),  # [NODE_F, C]
               dr64=dr64.astype(f),
               outWT=np.ascontiguousarray(np.asarray(inputs["out_W"], f).T),
               outb=np.asarray(inputs["out_b"], f).reshape(1, OUT_F))
    for l in range(2):
        W = np.asarray(inputs[f"convW{l}"], f)       # [HC, CDIM]
        a_s = np.asarray(inputs[f"att_src{l}"], f)   # [H, CDIM]
        a_d = np.asarray(inputs[f"att_dst{l}"], f)
        Wh = W.reshape(H, CDIM, CDIM)
        Ws = np.einsum("hcf,hc->fh", Wh, a_s)        # [CDIM, H]
        Wd = np.einsum("hcf,hc->fh", Wh, a_d)
        out[f"wcomb{l}"] = np.concatenate([W.T, Ws, Wd], 1)   # [CDIM, 264]
        out[f"convb{l}"] = np.asarray(inputs[f"convb{l}"], f).reshape(1, CDIM)
        out[f"lng{l}"] = np.asarray(inputs[f"ln_g{l}"], f).reshape(1, CDIM)
        out[f"lnb{l}"] = np.asarray(inputs[f"ln_b{l}"], f).reshape(1, CDIM)

    per_core = []
    for k in range(NCORES):
        sl = slice(k * NPC, (k + 1) * NPC)
        per_core.append(dict(
            xTa=np.ascontiguousarray(xp[sl].T.astype(bf)),        # [32, NPC]
            batchc=np.ascontiguousarray(
                batchp[sl].reshape(NGROUP, P).T.astype(np.int32)),  # [P, 98]
        ))
    return out, per_core


# --------------------------------------------------------------------------
# numpy emulation of the device schedule (for validation, no HW)
# --------------------------------------------------------------------------

def _emulate(cfg, prep, shared, per_core_w):
    import ml_dtypes
    bf = ml_dtypes.bfloat16
    f = np.float32
    rec = np.zeros((NPAD, RECP), bf)
    vq = np.zeros((NCORES, NPC + 1, 8), bf)
    had = np.zeros((NCORES, NPC, CDIM), f)
    out = np.zeros((NCORES, NPC, OUT_F), np.float16)

    # phase1 layer 0: h0 = x @ nodeW.T + dr64[batch]
    for k in range(NCORES):
        xT = per_core_w[k]["xTa"].astype(f)       # [32, NPC]
        bc = per_core_w[k]["batchc"]              # [P, 98]
        h0 = xT.T @ shared["nodeWT"].astype(f) + shared["dr64"][
            bc.T.reshape(-1)]
        had[k] = h0

    def phase1(l):
        for k in range(NCORES):
            prc = had[k] @ shared[f"wcomb{l}"]    # [NPC, 264]
            sl = slice(k * NPC, (k + 1) * NPC)
            rec[sl, 0:HC] = prc[:, 0:HC].astype(bf)
            rec[sl, HC:HC + H] = np.exp(prc[:, HC:HC + H]).astype(bf)
            rec[sl, HC + H:REC] = np.exp(0.2 * prc[:, HC:HC + H]).astype(bf)
            vq[k, :NPC, 0:4] = np.exp(prc[:, HC + H:REC]).astype(bf)
            vq[k, :NPC, 4:8] = np.exp(0.2 * prc[:, HC + H:REC]).astype(bf)

    def phase2(l):
        for k in range(NCORES):
            pc = prep["per_core"][k]
            col0 = 0
            for g in range(NGROUP):
                CH = cfg.chg[g]
                s = np.arange(CH * P)
                idx = pc["idx16"][(col0 * P + s) % 16,
                                  (col0 * P + s) // 16].astype(np.int64)
                # add per-call window bases
                cb_arr = np.asarray(cfg.callbase[g], np.int64)
                idx += cb_arr[s // 256]
                vqi = pc["vqidx16"][(col0 * P + s) % 16,
                                    (col0 * P + s) // 16].astype(np.int64)
                ds = pc["dstslot8"][:, col0:col0 + CH].astype(np.int64)
                rect = rec[idx].reshape(CH, P, RECP).transpose(1, 0, 2)
                vqt = vq[k][vqi].reshape(CH, P, 8).transpose(1, 0, 2)
                t1 = rect[:, :, HC:HC + H].astype(f) * vqt[:, :, 0:4].astype(f)
                t2 = rect[:, :, HC + H:REC].astype(f) * vqt[:, :, 4:8].astype(f)
                ex = np.maximum(t1, t2).astype(bf).astype(f)  # [P, CH, 4]
                xh = rect[:, :, 0:HC].astype(f).reshape(P, CH, H, CDIM)
                v = (xh * ex[:, :, :, None]).astype(bf).astype(f)
                M = (ds[:, :, None] == np.arange(P)[None, None, :])  # [P,CH,P]
                Mb = M.astype(bf).astype(f)
                # contract edges: pg[d, 0:256] / s4[d, 4]
                vflat = v.reshape(P, CH, HC)
                pg = np.einsum("ecd,ecf->df", Mb, vflat)
                s4 = np.einsum("ecd,ech->dh", Mb, ex)
                r4 = 1.0 / (s4 + 1e-16) / H
                yt = np.zeros((P, CDIM), f)
                for h_ in range(H):
                    yt += pg[:, h_ * CDIM:(h_ + 1) * CDIM] * r4[:, h_:h_ + 1]
                yt += shared[f"convb{l}"]
                mu = yt.mean(1, keepdims=True)
                var = ((yt - mu) ** 2).mean(1, keepdims=True)
                yt = (yt - mu) / np.sqrt(var + LN_EPS)
                yt = yt * shared[f"lng{l}"] + shared[f"lnb{l}"]
                yt = np.maximum(yt, 0.0)
                rows = slice(g * P, (g + 1) * P)
                yt2 = yt + had[k][rows]
                if l == 0:
                    had[k][rows] = yt2
                else:
                    out[k, rows] = (yt2 @ shared["outWT"]
                                    + shared["outb"]).astype(np.float16)
                col0 += CH

    phase1(0); phase2(0); phase1(1); phase2(1)
    return out


# --------------------------------------------------------------------------
# bass kernel
# --------------------------------------------------------------------------

def _blob_spec(cols):
    """(name, shape, dtype-name) for every section of the packed input blob,
    in order. All sections are 4-byte aligned."""
    return [
        ("xTa", [NODE_F, NPC], "bfloat16"),
        ("batchc", [P, NGROUP], "int32"),
        ("dr64", [G, CDIM], "float32"),
        ("nodeWT", [NODE_F, CDIM], "bfloat16"),
        ("wcomb0", [CDIM, REC], "float32"),
        ("wcomb1", [CDIM, REC], "float32"),
        ("convb0", [1, CDIM], "float32"),
        ("convb1", [1, CDIM], "float32"),
        ("lng0", [1, CDIM], "float32"),
        ("lng1", [1, CDIM], "float32"),
        ("lnb0", [1, CDIM], "float32"),
        ("lnb1", [1, CDIM], "float32"),
        ("outWT", [CDIM, OUT_F], "float32"),
        ("outb", [1, OUT_F], "float32"),
        ("idx16", [16, cols * 8], "int16"),
        ("vqidx16", [16, cols * 8], "int16"),
        ("dstslot8", [P, cols], "int8"),
    ]


def _blob_layout(cols):
    import ml_dtypes
    dtmap = dict(bfloat16=ml_dtypes.bfloat16, float32=np.float32,
                 int32=np.int32, int16=np.int16, int8=np.int8)
    off = 0
    lay = {}
    for name, sh, dtn in _blob_spec(cols):
        nb = int(np.prod(sh)) * np.dtype(dtmap[dtn]).itemsize
        lay[name] = (off, sh, dtmap[dtn], nb)
        off += (nb + 3) // 4 * 4
    return lay, off


def _build(cfg):
    import concourse.bass as bass
    import concourse.bacc as bacc
    import concourse.tile as tile
    from concourse import mybir
    from concourse.masks import make_identity

    f32 = mybir.dt.float32
    f16 = mybir.dt.float16
    i32 = mybir.dt.int32
    i16 = mybir.dt.int16
    bf16 = mybir.dt.bfloat16
    Alu = mybir.AluOpType
    Act = mybir.ActivationFunctionType

    CHMAX, cols = cfg.chmax, cfg.cols

    nc = bacc.Bacc("TRN2", target_bir_lowering=False, debug=False,
                   num_devices=NCORES)

    lay, blob_bytes = _blob_layout(cols)
    blob_d = nc.dram_tensor("blob", [1, blob_bytes], mybir.dt.int8,
                            kind="ExternalInput")

    unpack_jobs = []

    def unpack(nm, dt):
        off, sh, _, nb = lay[nm]
        t = nc.dram_tensor(nm, list(sh), dt)
        view = blob_d[0:1, off:off + nb].bitcast(dt).rearrange(
            "a (r c) -> (a r) c", r=sh[0])
        unpack_jobs.append((t, view))
        return t

    xTa_d = unpack("xTa", bf16)
    batchc_d = unpack("batchc", i32)
    dr64_d = unpack("dr64", f32)
    nodeWT_d = unpack("nodeWT", bf16)
    wcomb_d = [unpack(f"wcomb{l}", f32) for l in range(2)]
    convb_d = [unpack(f"convb{l}", f32) for l in range(2)]
    lng_d = [unpack(f"lng{l}", f32) for l in range(2)]
    lnb_d = [unpack(f"lnb{l}", f32) for l in range(2)]
    outWT_d = unpack("outWT", f32)
    outb_d = unpack("outb", f32)
    idx16_d = unpack("idx16", i16)
    vqidx16_d = unpack("vqidx16", i16)
    dstslot8_d = unpack("dstslot8", mybir.dt.int8)

    out_d = nc.dram_tensor("out", [NPC, OUT_F], f16, kind="ExternalOutput")

    rec_own_d = nc.dram_tensor("rec_own", [NPC, RECP], bf16)
    rec_d = nc.dram_tensor("rec", [NPAD, RECP], bf16, addr_space="Shared")
    vq_d = nc.dram_tensor("vq", [NPC + 1, VQW], bf16)
    idx128_d = nc.dram_tensor("idx128", [P, cols * 8], i16)
    vqidx128_d = nc.dram_tensor("vqidx128", [P, cols * 8], i16)

    from contextlib import ExitStack
    with tile.TileContext(nc) as tc, ExitStack() as ctx:
        cpool = ctx.enter_context(tc.tile_pool(name="const", bufs=1))
        p1 = ctx.enter_context(tc.tile_pool(name="p1", bufs=2))
        p2 = ctx.enter_context(tc.tile_pool(name="p2", bufs=2))

        for t, view in unpack_jobs:
            nc.sync.dma_start(out=t[:, :], in_=view)

        def cload(dram):
            t = cpool.tile(list(dram.shape), dram.dtype, tag=f"c_{dram.name}")
            nc.sync.dma_start(out=t[:], in_=dram[:])
            return t

        xTa_sb = cload(xTa_d)
        batchc_sb = cload(batchc_d)
        nodeWT_sb = cload(nodeWT_d)
        wcomb_sb = [cload(d) for d in wcomb_d]
        outWT_sb = cload(outWT_d)

        ds8_sb = cload(dstslot8_d)

        # replicate [16, X] index streams to [128, X] in DRAM
        for k in range(8):
            nc.sync.dma_start(out=idx128_d[k * 16:(k + 1) * 16, :],
                              in_=idx16_d[:, :])
            nc.sync.dma_start(out=vqidx128_d[k * 16:(k + 1) * 16, :],
                              in_=vqidx16_d[:, :])

        iota_sb = cpool.tile([P, P], i32)
        nc.gpsimd.iota(iota_sb[:], pattern=[[1, P]], base=0,
                       channel_multiplier=0)
        iota8_sb = cpool.tile([P, P], mybir.dt.int8)
        nc.vector.tensor_copy(iota8_sb[:], iota_sb[:])
        ident_sb = cpool.tile([P, P], f32)
        make_identity(nc, ident_sb[:])

        # broadcast [1, C] channel weights to all partitions via outer
        # product with a ones column
        ones_sb = cpool.tile([1, P], f32)
        nc.vector.memset(ones_sb[:], 1.0)

        def crep(dram, width):
            row = cpool.tile([1, width], f32, tag=f"r_{dram.name}")
            nc.sync.dma_start(out=row[:], in_=dram[:])
            t = cpool.tile([P, width], f32, tag=f"b_{dram.name}")
            with tc.tile_pool(name=f"pb_{dram.name}", bufs=1,
                              space="PSUM") as pb:
                pt = pb.tile([P, width], f32)
                nc.tensor.matmul(pt[:], lhsT=ones_sb[:], rhs=row[:],
                                 start=True, stop=True)
                nc.scalar.copy(t[:], pt[:])
            return t

        convb_sb = [crep(d, CDIM) for d in convb_d]
        lng_sb = [crep(d, CDIM) for d in lng_d]
        lnb_sb = [crep(d, CDIM) for d in lnb_d]
        outb_sb = crep(outb_d, OUT_F)

        # zero row NPC of the vq table (pad-slot target)
        zrow_sb = cpool.tile([1, VQW], bf16)
        nc.vector.memset(zrow_sb[:], 0.0)
        nc.sync.dma_start(out=vq_d[NPC:NPC + 1, :], in_=zrow_sb[:])

        # h rows (residual / layer-1 input), SBUF-resident
        had_sb = cpool.tile([P, NGROUP, CDIM], f32, tag="had")

        # ------------------------------------------------------------------
        def phase1(l):
            with tc.tile_pool(name=f"ps1_{l}", bufs=2, space="PSUM") as pp:
                for b0 in range(0, NGROUP, TB):
                    recb = p1.tile([P, TB, RECP], bf16, tag="recb")
                    vqb = p1.tile([P, TB, 8], bf16, tag="vqb")
                    for tt in range(TB):
                        t = b0 + tt
                        if l == 0:
                            ph = pp.tile([P, CDIM], f32, tag="ph")
                            nc.tensor.matmul(ph[:],
                                             lhsT=xTa_sb[:, t * P:(t + 1) * P],
                                             rhs=nodeWT_sb[:], start=True,
                                             stop=True)
                            drt = p1.tile([P, CDIM], f32, tag="drt")
                            nc.gpsimd.indirect_dma_start(
                                out=drt[:], out_offset=None, in_=dr64_d[:],
                                in_offset=bass.IndirectOffsetOnAxis(
                                    ap=batchc_sb[:, t:t + 1], axis=0))
                            nc.vector.tensor_add(had_sb[:, t, :], ph[:],
                                                 drt[:])
                        pt = pp.tile([CDIM, P], f32, tag="pt")
                        nc.tensor.transpose(pt[:], had_sb[:, t, :],
                                            ident_sb[:])
                        hT = p1.tile([CDIM, P], f32, tag="hT")
                        nc.scalar.copy(hT[:], pt[:])
                        prc = pp.tile([P, REC], f32, tag="prc")
                        nc.tensor.matmul(prc[:], lhsT=hT[:],
                                         rhs=wcomb_sb[l][:], start=True,
                                         stop=True)
                        nc.scalar.copy(recb[:, tt, 0:HC], prc[:, 0:HC])
                        nc.scalar.activation(recb[:, tt, HC:HC + H],
                                             prc[:, HC:HC + H], Act.Exp)
                        nc.scalar.activation(recb[:, tt, HC + H:REC],
                                             prc[:, HC:HC + H], Act.Exp,
                                             scale=0.2)
                        nc.scalar.activation(vqb[:, tt, 0:4],
                                             prc[:, HC + H:REC], Act.Exp)
                        nc.scalar.activation(vqb[:, tt, 4:8],
                                             prc[:, HC + H:REC], Act.Exp,
                                             scale=0.2)
                    r0 = b0 * P
                    rows = TB * P
                    nc.sync.dma_start(
                        out=rec_own_d[r0:r0 + rows, :].rearrange(
                            "(c p) f -> p c f", p=P),
                        in_=recb[:, :, :])
                    nc.sync.dma_start(
                        out=vq_d[r0:r0 + rows, 0:8].rearrange(
                            "(c p) f -> p c f", p=P),
                        in_=vqb[:, :, :])
            nc.gpsimd.collective_compute(
                "AllGather", mybir.AluOpType.bypass,
                replica_groups=[list(range(NCORES))],
                ins=[rec_own_d[:, :].opt()],
                outs=[rec_d[:, :].opt()])

        # ------------------------------------------------------------------
        def phase2(l):
            with tc.tile_pool(name=f"ps2_{l}", bufs=2, space="PSUM") as pp:
                col0 = 0
                for g in range(NGROUP):
                    CH = cfg.chg[g]
                    idxt = p2.tile([P, CHMAX * 8], i16, tag="idxt")
                    nc.sync.dma_start(out=idxt[:, :CH * 8],
                                      in_=idx128_d[:, col0 * 8:(col0 + CH) * 8])
                    vqit = p2.tile([P, CHMAX * 8], i16, tag="vqit")
                    nc.sync.dma_start(
                        out=vqit[:, :CH * 8],
                        in_=vqidx128_d[:, col0 * 8:(col0 + CH) * 8])
                    rect = p2.tile([P, CHMAX, RECP], bf16, tag="rect")
                    done = 0
                    i = 0
                    while done < CH:   # HW envelope: <=256 idxs per call
                        st = min(2, CH - done)
                        base = cfg.callbase[g][i]
                        nrows = min(BUCKET, NPAD - base)
                        nc.gpsimd.dma_gather(
                            rect[:, done:done + st, :],
                            rec_d[base:base + nrows, :],
                            idxt[:, done * 8:(done + st) * 8],
                            st * P, st * P, RECP)
                        done += st
                        i += 1
                    vqt = p2.tile([P, CHMAX, VQW], bf16, tag="vqt")
                    done = 0
                    while done < CH:
                        st = min(2, CH - done)
                        nc.gpsimd.dma_gather(
                            vqt[:, done:done + st, :], vq_d[0:NPC + 1, :],
                            vqit[:, done * 8:(done + st) * 8],
                            st * P, st * P, VQW)
                        done += st
                    # one-hot M[edge, dst_slot]
                    Mt = p2.tile([P, CHMAX, P], bf16, tag="Mt")
                    nc.vector.tensor_tensor(
                        Mt[:, :CH, :],
                        ds8_sb[:, col0:col0 + CH][:, :, None].to_broadcast(
                            [P, CH, P]),
                        iota8_sb[:, None, :].to_broadcast([P, CH, P]),
                        Alu.is_equal)
                    # ex = max(u*v, p*q) -> rect[..., 256:260]
                    t1 = p2.tile([P, CHMAX, H], f32, tag="t1")
                    nc.vector.tensor_tensor(t1[:, :CH, :],
                                            rect[:, :CH, HC:HC + H],
                                            vqt[:, :CH, 0:4], Alu.mult)
                    t2 = p2.tile([P, CHMAX, H], f32, tag="t2")
                    nc.vector.tensor_tensor(t2[:, :CH, :],
                                            rect[:, :CH, HC + H:REC],
                                            vqt[:, :CH, 4:8], Alu.mult)
                    nc.vector.tensor_tensor(rect[:, :CH, HC:HC + H],
                                            t1[:, :CH, :], t2[:, :CH, :],
                                            Alu.max)
                    # V = ex * xh (per head, in place)
                    for h_ in range(H):
                        nc.vector.tensor_tensor(
                            rect[:, :CH, h_ * CDIM:(h_ + 1) * CDIM],
                            rect[:, :CH, h_ * CDIM:(h_ + 1) * CDIM],
                            rect[:, :CH, HC + h_:HC + h_ + 1].to_broadcast(
                                [P, CH, CDIM]),
                            Alu.mult)
                    # contract over edges: pg[:, 0:256]=sum ex*xh, [256:260]=s
                    pg = pp.tile([P, HC + H], f32, tag="pg")
                    for c in range(CH):
                        nc.tensor.matmul(pg[:], lhsT=Mt[:, c, :],
                                         rhs=rect[:, c, 0:HC + H],
                                         start=(c == 0), stop=(c == CH - 1))
                    # r = 1 / (s + eps) / H
                    s4 = p2.tile([P, H], f32, tag="s4")
                    nc.vector.tensor_scalar(s4[:], pg[:, HC:HC + H], 1e-16,
                                            None, Alu.add)
                    r4 = p2.tile([P, H], f32, tag="r4")
                    nc.vector.reciprocal(r4[:], s4[:])
                    nc.vector.tensor_scalar_mul(r4[:], r4[:], 1.0 / H)
                    # head mean
                    yt = p2.tile([P, CDIM], f32, tag="yt")
                    tmp = p2.tile([P, CDIM], f32, tag="tmp")
                    nc.vector.tensor_scalar(yt[:], pg[:, 0:CDIM], r4[:, 0:1],
                                            None, Alu.mult)
                    for h_ in range(1, H):
                        nc.vector.tensor_scalar(
                            tmp[:], pg[:, h_ * CDIM:(h_ + 1) * CDIM],
                            r4[:, h_:h_ + 1], None, Alu.mult)
                        nc.vector.tensor_add(yt[:], yt[:], tmp[:])
                    nc.vector.tensor_add(yt[:], yt[:], convb_sb[l][:])
                    # layernorm
                    mu = p2.tile([P, 1], f32, tag="mu")
                    nc.vector.tensor_reduce(mu[:], yt[:], mybir.AxisListType.X,
                                            Alu.add)
                    nc.vector.tensor_scalar_mul(mu[:], mu[:], 1.0 / CDIM)
                    nc.vector.tensor_scalar(yt[:], yt[:], mu[:, 0:1], None,
                                            Alu.subtract)
                    sq = p2.tile([P, CDIM], f32, tag="sq")
                    var = p2.tile([P, 1], f32, tag="var")
                    nc.scalar.activation(sq[:], yt[:], Act.Square,
                                         accum_out=var[:])
                    nc.vector.tensor_scalar(var[:], var[:], 1.0 / CDIM,
                                            LN_EPS, Alu.mult, Alu.add)
                    sd = p2.tile([P, 1], f32, tag="sd")
                    nc.scalar.sqrt(sd[:], var[:])
                    inv = p2.tile([P, 1], f32, tag="inv")
                    nc.vector.reciprocal(inv[:], sd[:])
                    nc.vector.tensor_scalar(yt[:], yt[:], inv[:, 0:1], None,
                                            Alu.mult)
                    nc.vector.tensor_mul(yt[:], yt[:], lng_sb[l][:])
                    nc.vector.tensor_add(yt[:], yt[:], lnb_sb[l][:])
                    nc.vector.tensor_scalar_max(yt[:], yt[:], 0.0)
                    # residual
                    if l == 0:
                        nc.vector.tensor_add(had_sb[:, g, :], yt[:],
                                             had_sb[:, g, :])
                    else:
                        yt2 = p2.tile([P, CDIM], f32, tag="yt2")
                        nc.vector.tensor_add(yt2[:], yt[:], had_sb[:, g, :])
                        pt2 = pp.tile([CDIM, P], f32, tag="pt2")
                        nc.tensor.transpose(pt2[:], yt2[:], ident_sb[:])
                        hT2 = p2.tile([CDIM, P], f32, tag="hT2")
                        nc.scalar.copy(hT2[:], pt2[:])
                        po = pp.tile([P, OUT_F], f32, tag="po")
                        nc.tensor.matmul(po[:], lhsT=hT2[:], rhs=outWT_sb[:],
                                         start=True, stop=True)
                        ot = p2.tile([P, OUT_F], f16, tag="ot")
                        nc.vector.tensor_add(ot[:], po[:], outb_sb[:])
                        nc.sync.dma_start(out=out_d[g * P:(g + 1) * P, :],
                                          in_=ot[:])
                    col0 += CH

        # ------------------------------------------------------------------
        phase1(0)
        phase2(0)
        phase1(1)
        phase2(1)

    nc.compile()
    return nc


# --------------------------------------------------------------------------
# entry point
# --------------------------------------------------------------------------

def _in_maps(cfg, prep, shared, per_core_w):
    lay, blob_bytes = _blob_layout(cfg.cols)
    maps = []
    for k in range(NCORES):
        src = dict(shared)
        src.update(prep["per_core"][k])
        src.update(per_core_w[k])
        blob = np.zeros((1, blob_bytes), np.int8)
        for nm, (off, sh, dt, nb) in lay.items():
            a = np.ascontiguousarray(np.asarray(src[nm], dt))
            assert list(a.shape) == list(sh), (nm, a.shape, sh)
            blob[0, off:off + nb] = a.view(np.int8).ravel()
        maps.append(dict(blob=blob))
    return maps


def _gather_out(prep, results):
    out = np.empty((N, OUT_F), np.float32)
    order = prep["order"]
    for k in range(NCORES):
        blk = order[k * NPC:(k + 1) * NPC]
        valid = blk >= 0
        out[blk[valid]] = results[k]["out"][valid].astype(np.float32)
    return out


def kernel(**inputs):
    edge_index = np.asarray(inputs["edge_index"])
    prep = _host_prep(edge_index)
    cfg = prep["cfg"]
    shared, per_core_w = _host_weights(inputs, prep["order"])
    nc = _build(cfg)
    maps = _in_maps(cfg, prep, shared, per_core_w)

    from concourse import bass_utils
    res = bass_utils.run_bass_kernel_spmd(nc, maps,
                                          core_ids=list(range(NCORES)))
    return _gather_out(prep, res.results)


# revision 43
# speedup vs baseline: 1.3983x; 1.1973x over previous
"""GAT (2-layer, 4-head, segment-softmax) message-passing kernel for 8 Trainium2
NeuronCores.

Strategy (dst-sharded, edge aggregation as one-hot matmuls, factored softmax):
  * Nodes are degree-strided across 784 groups of 128 slots (8 cores x 98
    groups) so every group carries a near-equal edge load; the permutation is
    (core, group, slot) order.
  * Per layer, each core computes records only for its OWN nodes
    (rec[n_own] = [xh(256) | u=exp(a_s)(4) | p=exp(0.2 a_s)(4) | pad], bf16)
    plus a dst-side table vq[n_own] = [v=exp(a_d) | q=exp(0.2 a_d)]; an
    8-core AllGather assembles the full record table.
  * exp(lrelu(a_s+a_d)) == max(u*v, p*q) exactly (exp is monotone), so the
    per-edge attention numerator needs only elementwise ops on gathered
    values - no per-chunk transpose/matmul broadcast.
  * For each destination group, the core gathers the in-edges' source
    records with gpsimd dma_gather (int16 indices, 32768-row buckets) and
    the dst-side vq rows (single bucket, local), builds the one-hot
    incidence M[edge, dst_slot] on the vector engine, and reduces both the
    softmax denominators and weighted feature sums with PSUM-accumulated
    matmuls contracting over edges. Softmax normalization is applied on the
    dst side after the reduction (the max-subtraction of the reference is a
    denominator-cancelling no-op at these magnitudes).
  * Head-mean + LayerNorm + ReLU + residual run per group on vector/scalar
    engines; h stays resident in SBUF between layers; the final projection
    is fused into layer 2's group loop.
"""

import sys

sys.path.insert(0, "/opt/trn_rl_repo")

import numpy as np

# ---- problem constants (hardcoded; kernel.py must be self-contained) ----
N = 100000
E = 1600000
G = 64
H = 4
CDIM = 64
NODE_F = 32
DRONE_F = 16
OUT_F = 32
LN_EPS = 1e-5
NCORES = 8
P = 128
HC = H * CDIM          # 256
REC = HC + 2 * H       # 264: [xh(256) | u(4) | p(4)]
RECP = 384             # padded record elems (bf16 row = 768B, mult of 256)
VQW = 128              # vq table row elems (bf16 row = 256B)
NGROUP = 98
NPC = NGROUP * P       # 12544 padded rows per core
NPAD = NCORES * NPC    # 100352
BUCKET = 32768
NBUCKETS = -(-NPAD // BUCKET)  # 4
TB = 7                 # phase-1 tile batch (98 = 14*7)


class _Cfg:
    def __init__(self, chg, callbase):
        self.chg = chg                # chunks per group (shared across cores)
        self.callbase = callbase      # [NGROUP][ncalls] gather-window bases
        self.chmax = max(chg)
        self.cols = sum(chg)


# --------------------------------------------------------------------------
# host-side preprocessing
# --------------------------------------------------------------------------

def _host_prep(edge_index):
    """Degree-strided node permutation + per-core int16 gather streams."""
    loop = np.arange(N, dtype=np.int64)
    src = np.concatenate([edge_index[0].astype(np.int64), loop])
    dst = np.concatenate([edge_index[1].astype(np.int64), loop])
    deg = np.bincount(dst, minlength=N)

    # node rank i (by degree desc) -> grp784 = i % 784, slot = i // 784
    order_by_deg = np.argsort(-deg, kind="stable")
    NG = NCORES * NGROUP
    grp784 = np.arange(N) % NG
    slot784 = np.arange(N) // NG
    core_of_g = grp784 % NCORES
    group_of_g = grp784 // NCORES
    pos_of = np.empty(N, np.int64)            # node -> padded global position
    pos_of[order_by_deg] = core_of_g * NPC + group_of_g * P + slot784
    order = np.full(NPAD, -1, np.int64)       # padded position -> node
    order[pos_of] = np.arange(N)

    e_core = pos_of[dst] // NPC
    e_group = (pos_of[dst] % NPC) // P
    e_slot = pos_of[dst] % P

    # per-core edge streams sorted by (group, src position); chunk schedule
    # shared across cores: chg[g] = max_k ceil(E_gk / 128). Gather windows
    # are per 256-idx call (2 chunks), base = min over cores of the call's
    # first src position (sliding window; no bucket-grid round-up).
    streams = []
    cnt_kg = np.zeros((NCORES, NGROUP), np.int64)
    for k in range(NCORES):
        mask = e_core == k
        es = pos_of[src[mask]]
        eg = e_group[mask]
        esl = e_slot[mask]
        o = np.lexsort((es, eg))
        es, eg, esl = es[o], eg[o], esl[o]
        np.add.at(cnt_kg[k], eg, 1)
        streams.append((es, eg, esl))
    chg_np = -(-cnt_kg.max(axis=0) // P)                   # [NGROUP]
    chg = [int(c) for c in chg_np]
    goff = np.concatenate([[0], np.cumsum(chg_np)])[:-1]
    ncalls = -(-chg_np // 2)
    cumcalls = np.concatenate([[0], np.cumsum(ncalls)])
    tot_calls = int(cumcalls[-1])

    # call bases: min over cores of first src pos in each call's edge range
    first = np.full((NCORES, tot_calls), np.iinfo(np.int64).max, np.int64)
    last = np.zeros((NCORES, tot_calls), np.int64)
    for k in range(NCORES):
        es, eg, esl = streams[k]
        gstart = np.concatenate([[0], np.cumsum(cnt_kg[k])])[:-1]
        j = np.arange(len(es)) - gstart[eg]
        call = cumcalls[eg] + j // 256
        np.minimum.at(first[k], call, es)
        np.maximum.at(last[k], call, es)
    callbase_flat = first.min(axis=0)
    empty = callbase_flat == np.iinfo(np.int64).max
    callbase_flat[empty] = 0
    span = last.max(axis=0) - callbase_flat
    assert span.max() < BUCKET, f"gather window overflow: {span.max()}"
    callbase = [[int(callbase_flat[cumcalls[g] + i]) for i in range(ncalls[g])]
                for g in range(NGROUP)]
    cfg = _Cfg(chg, callbase)
    cols = cfg.cols

    per_core = []
    for k in range(NCORES):
        es, eg, esl = streams[k]
        gstart = np.concatenate([[0], np.cumsum(cnt_kg[k])])[:-1]
        j = np.arange(len(es)) - gstart[eg]
        slotj = goff[eg] * P + j              # global edge slot in stream
        call = cumcalls[eg] + j // 256
        rel = es - callbase_flat[call]

        idx16 = np.zeros((16, cols * 8), np.int16)
        idx16[slotj % 16, slotj // 16] = rel
        # dst slot streams, natural layout (one-hot) and 16-wrapped (vq
        # gather indices, derived on device as g*128 + slot + 1 into the
        # 1-shifted vq table whose row 0 is zeroed). Pad slots carry -1:
        # never one-hot matched, and they index a dont-care vq row.
        dstslot8 = np.full((P, cols), -1, np.int8)
        dstslot8[slotj % P, slotj // P] = esl
        dsw8 = np.full((16, cols * 8), -1, np.int8)
        dsw8[slotj % 16, slotj // 16] = esl
        per_core.append(dict(idx16=idx16, dsw8=dsw8, dstslot8=dstslot8))
    return dict(order=order, pos_of=pos_of, cfg=cfg, per_core=per_core)


def _host_weights(inputs, order):
    """Permuted inputs + combined weights (f32/bf16)."""
    import ml_dtypes
    f = np.float32
    bf = ml_dtypes.bfloat16
    valid = order >= 0
    xp = np.zeros((NPAD, NODE_F), f)
    xp[valid] = np.asarray(inputs["x"], f)[order[valid]]
    batchp = np.zeros(NPAD, np.int32)
    batchp[valid] = np.asarray(inputs["batch"], np.int32)[order[valid]]

    dr64 = (np.asarray(inputs["drone_feat"], f) @ np.asarray(inputs["drone_W"], f).T
            + np.asarray(inputs["drone_b"], f) + np.asarray(inputs["node_b"], f))

    out = dict(nodeWT=np.ascontiguousarray(
                   np.asarray(inputs["node_W"], f).T.astype(bf)),  # [NODE_F, C]
               dr64=dr64.astype(f),
               outWT=np.ascontiguousarray(np.asarray(inputs["out_W"], f).T),
               outb=np.asarray(inputs["out_b"], f).reshape(1, OUT_F))
    for l in range(2):
        W = np.asarray(inputs[f"convW{l}"], f)       # [HC, CDIM]
        a_s = np.asarray(inputs[f"att_src{l}"], f)   # [H, CDIM]
        a_d = np.asarray(inputs[f"att_dst{l}"], f)
        Wh = W.reshape(H, CDIM, CDIM)
        Ws = np.einsum("hcf,hc->fh", Wh, a_s)        # [CDIM, H]
        Wd = np.einsum("hcf,hc->fh", Wh, a_d)
        out[f"wcomb{l}"] = np.concatenate([W.T, Ws, Wd], 1)   # [CDIM, 264]
        out[f"convb{l}"] = np.asarray(inputs[f"convb{l}"], f).reshape(1, CDIM)
        out[f"lng{l}"] = np.asarray(inputs[f"ln_g{l}"], f).reshape(1, CDIM)
        out[f"lnb{l}"] = np.asarray(inputs[f"ln_b{l}"], f).reshape(1, CDIM)

    per_core = []
    for k in range(NCORES):
        sl = slice(k * NPC, (k + 1) * NPC)
        per_core.append(dict(
            xTa=np.ascontiguousarray(xp[sl].T.astype(bf)),        # [32, NPC]
            batchc=np.ascontiguousarray(
                batchp[sl].reshape(NGROUP, P).T.astype(np.int8)),  # [P, 98]
        ))
    return out, per_core


# --------------------------------------------------------------------------
# numpy emulation of the device schedule (for validation, no HW)
# --------------------------------------------------------------------------

def _emulate(cfg, prep, shared, per_core_w):
    import ml_dtypes
    bf = ml_dtypes.bfloat16
    f = np.float32
    rec = np.zeros((NPAD, RECP), bf)
    vq = np.zeros((NCORES, NPC + 1, 8), bf)   # row 0 zeroed; node p -> row p+1
    had = np.zeros((NCORES, NPC, CDIM), f)
    out = np.zeros((NCORES, NPC, OUT_F), np.float16)

    # phase1 layer 0: h0 = x @ nodeW.T + dr64[batch]
    for k in range(NCORES):
        xT = per_core_w[k]["xTa"].astype(f)       # [32, NPC]
        bc = per_core_w[k]["batchc"]              # [P, 98]
        h0 = xT.T @ shared["nodeWT"].astype(f) + shared["dr64"][
            bc.T.reshape(-1)]
        had[k] = h0

    def phase1(l):
        for k in range(NCORES):
            prc = had[k] @ shared[f"wcomb{l}"]    # [NPC, 264]
            sl = slice(k * NPC, (k + 1) * NPC)
            rec[sl, 0:HC] = prc[:, 0:HC].astype(bf)
            rec[sl, HC:HC + H] = np.exp(prc[:, HC:HC + H]).astype(bf)
            rec[sl, HC + H:REC] = np.exp(0.2 * prc[:, HC:HC + H]).astype(bf)
            vq[k, 1:NPC + 1, 0:4] = np.exp(prc[:, HC + H:REC]).astype(bf)
            vq[k, 1:NPC + 1, 4:8] = np.exp(0.2 * prc[:, HC + H:REC]).astype(bf)

    def phase2(l):
        for k in range(NCORES):
            pc = prep["per_core"][k]
            col0 = 0
            for g in range(NGROUP):
                CH = cfg.chg[g]
                s = np.arange(CH * P)
                idx = pc["idx16"][(col0 * P + s) % 16,
                                  (col0 * P + s) // 16].astype(np.int64)
                # add per-call window bases
                cb_arr = np.asarray(cfg.callbase[g], np.int64)
                idx += cb_arr[s // 256]
                vqi = pc["dsw8"][(col0 * P + s) % 16,
                                 (col0 * P + s) // 16].astype(np.int64) \
                    + g * P + 1
                ds = pc["dstslot8"][:, col0:col0 + CH].astype(np.int64)
                rect = rec[idx].reshape(CH, P, RECP).transpose(1, 0, 2)
                vqt = vq[k][vqi].reshape(CH, P, 8).transpose(1, 0, 2)
                t1 = rect[:, :, HC:HC + H].astype(f) * vqt[:, :, 0:4].astype(f)
                t2 = rect[:, :, HC + H:REC].astype(f) * vqt[:, :, 4:8].astype(f)
                ex = np.maximum(t1, t2).astype(bf).astype(f)  # [P, CH, 4]
                xh = rect[:, :, 0:HC].astype(f).reshape(P, CH, H, CDIM)
                v = (xh * ex[:, :, :, None]).astype(bf).astype(f)
                M = (ds[:, :, None] == np.arange(P)[None, None, :])  # [P,CH,P]
                Mb = M.astype(bf).astype(f)
                # contract edges: pg[d, 0:256] / s4[d, 4]
                vflat = v.reshape(P, CH, HC)
                pg = np.einsum("ecd,ecf->df", Mb, vflat)
                s4 = np.einsum("ecd,ech->dh", Mb, ex)
                r4 = 1.0 / (s4 + 1e-16) / H
                yt = np.zeros((P, CDIM), f)
                for h_ in range(H):
                    yt += pg[:, h_ * CDIM:(h_ + 1) * CDIM] * r4[:, h_:h_ + 1]
                yt += shared[f"convb{l}"]
                mu = yt.mean(1, keepdims=True)
                var = ((yt - mu) ** 2).mean(1, keepdims=True)
                yt = (yt - mu) / np.sqrt(var + LN_EPS)
                yt = yt * shared[f"lng{l}"] + shared[f"lnb{l}"]
                yt = np.maximum(yt, 0.0)
                rows = slice(g * P, (g + 1) * P)
                yt2 = yt + had[k][rows]
                if l == 0:
                    had[k][rows] = yt2
                else:
                    out[k, rows] = (yt2 @ shared["outWT"]
                                    + shared["outb"]).astype(np.float16)
                col0 += CH

    phase1(0); phase2(0); phase1(1); phase2(1)
    return out


# --------------------------------------------------------------------------
# bass kernel
# --------------------------------------------------------------------------

def _blob_spec(cols):
    """(name, shape, dtype-name) for every section of the packed input blob,
    in order. All sections are 4-byte aligned."""
    return [
        ("xTa", [NODE_F, NPC], "bfloat16"),
        ("batchc", [P, NGROUP], "int8"),
        ("dr64", [G, CDIM], "float32"),
        ("nodeWT", [NODE_F, CDIM], "bfloat16"),
        ("wcomb0", [CDIM, REC], "float32"),
        ("wcomb1", [CDIM, REC], "float32"),
        ("convb0", [1, CDIM], "float32"),
        ("convb1", [1, CDIM], "float32"),
        ("lng0", [1, CDIM], "float32"),
        ("lng1", [1, CDIM], "float32"),
        ("lnb0", [1, CDIM], "float32"),
        ("lnb1", [1, CDIM], "float32"),
        ("outWT", [CDIM, OUT_F], "float32"),
        ("outb", [1, OUT_F], "float32"),
        ("idx16", [16, cols * 8], "int16"),
        ("dsw8", [16, cols * 8], "int8"),
        ("dstslot8", [P, cols], "int8"),
    ]


def _blob_layout(cols):
    import ml_dtypes
    dtmap = dict(bfloat16=ml_dtypes.bfloat16, float32=np.float32,
                 int32=np.int32, int16=np.int16, int8=np.int8)
    off = 0
    lay = {}
    for name, sh, dtn in _blob_spec(cols):
        nb = int(np.prod(sh)) * np.dtype(dtmap[dtn]).itemsize
        lay[name] = (off, sh, dtmap[dtn], nb)
        off += (nb + 3) // 4 * 4
    return lay, off


def _build(cfg):
    import concourse.bass as bass
    import concourse.bacc as bacc
    import concourse.tile as tile
    from concourse import mybir
    from concourse.masks import make_identity

    f32 = mybir.dt.float32
    f16 = mybir.dt.float16
    i32 = mybir.dt.int32
    i16 = mybir.dt.int16
    bf16 = mybir.dt.bfloat16
    Alu = mybir.AluOpType
    Act = mybir.ActivationFunctionType

    CHMAX, cols = cfg.chmax, cfg.cols

    nc = bacc.Bacc("TRN2", target_bir_lowering=False, debug=False,
                   num_devices=NCORES)

    lay, blob_bytes = _blob_layout(cols)
    blob_d = nc.dram_tensor("blob", [1, blob_bytes], mybir.dt.int8,
                            kind="ExternalInput")

    unpack_jobs = []

    def unpack(nm, dt):
        off, sh, _, nb = lay[nm]
        t = nc.dram_tensor(nm, list(sh), dt)
        view = blob_d[0:1, off:off + nb].bitcast(dt).rearrange(
            "a (r c) -> (a r) c", r=sh[0])
        unpack_jobs.append((t, view))
        return t

    i8 = mybir.dt.int8
    xTa_d = unpack("xTa", bf16)
    batchc_d = unpack("batchc", i8)
    dr64_d = unpack("dr64", f32)
    nodeWT_d = unpack("nodeWT", bf16)
    wcomb_d = [unpack(f"wcomb{l}", f32) for l in range(2)]
    convb_d = [unpack(f"convb{l}", f32) for l in range(2)]
    lng_d = [unpack(f"lng{l}", f32) for l in range(2)]
    lnb_d = [unpack(f"lnb{l}", f32) for l in range(2)]
    outWT_d = unpack("outWT", f32)
    outb_d = unpack("outb", f32)
    idx16_d = unpack("idx16", i16)
    dsw8_d = unpack("dsw8", i8)
    dstslot8_d = unpack("dstslot8", i8)

    out_d = nc.dram_tensor("out", [NPC, OUT_F], f16, kind="ExternalOutput")

    rec_own_d = nc.dram_tensor("rec_own", [NPC, RECP], bf16)
    rec_d = nc.dram_tensor("rec", [NPAD, RECP], bf16, addr_space="Shared")
    vq_d = nc.dram_tensor("vq", [NPC + 1, VQW], bf16)
    idx128_d = nc.dram_tensor("idx128", [P, cols * 8], i16)
    dsw128_d = nc.dram_tensor("dsw128", [P, cols * 8], i8)

    from contextlib import ExitStack
    with tile.TileContext(nc) as tc, ExitStack() as ctx:
        cpool = ctx.enter_context(tc.tile_pool(name="const", bufs=1))
        p1 = ctx.enter_context(tc.tile_pool(name="p1", bufs=2))
        p2 = ctx.enter_context(tc.tile_pool(name="p2", bufs=2))

        for t, view in unpack_jobs:
            nc.sync.dma_start(out=t[:, :], in_=view)

        def cload(dram):
            t = cpool.tile(list(dram.shape), dram.dtype, tag=f"c_{dram.name}")
            nc.sync.dma_start(out=t[:], in_=dram[:])
            return t

        xTa_sb = cload(xTa_d)
        batchc8_sb = cload(batchc_d)
        batchc_sb = cpool.tile([P, NGROUP], i32, tag="batchc32")
        nc.vector.tensor_copy(batchc_sb[:], batchc8_sb[:])
        nodeWT_sb = cload(nodeWT_d)
        wcomb_sb = [cload(d) for d in wcomb_d]
        outWT_sb = cload(outWT_d)

        ds8_sb = cload(dstslot8_d)

        # replicate [16, X] index streams to [128, X] in DRAM
        for k in range(8):
            nc.sync.dma_start(out=idx128_d[k * 16:(k + 1) * 16, :],
                              in_=idx16_d[:, :])
            nc.sync.dma_start(out=dsw128_d[k * 16:(k + 1) * 16, :],
                              in_=dsw8_d[:, :])

        iota_sb = cpool.tile([P, P], i32)
        nc.gpsimd.iota(iota_sb[:], pattern=[[1, P]], base=0,
                       channel_multiplier=0)
        iota8_sb = cpool.tile([P, P], mybir.dt.int8)
        nc.vector.tensor_copy(iota8_sb[:], iota_sb[:])
        ident_sb = cpool.tile([P, P], f32)
        make_identity(nc, ident_sb[:])

        # broadcast [1, C] channel weights to all partitions via outer
        # product with a ones column
        ones_sb = cpool.tile([1, P], f32)
        nc.vector.memset(ones_sb[:], 1.0)

        def crep(dram, width):
            row = cpool.tile([1, width], f32, tag=f"r_{dram.name}")
            nc.sync.dma_start(out=row[:], in_=dram[:])
            t = cpool.tile([P, width], f32, tag=f"b_{dram.name}")
            with tc.tile_pool(name=f"pb_{dram.name}", bufs=1,
                              space="PSUM") as pb:
                pt = pb.tile([P, width], f32)
                nc.tensor.matmul(pt[:], lhsT=ones_sb[:], rhs=row[:],
                                 start=True, stop=True)
                nc.scalar.copy(t[:], pt[:])
            return t

        convb_sb = [crep(d, CDIM) for d in convb_d]
        lng_sb = [crep(d, CDIM) for d in lng_d]
        lnb_sb = [crep(d, CDIM) for d in lnb_d]
        outb_sb = crep(outb_d, OUT_F)

        # zero row 0 of the 1-shifted vq table (group-0 pad-slot target)
        zrow_sb = cpool.tile([1, VQW], bf16)
        nc.vector.memset(zrow_sb[:], 0.0)
        nc.sync.dma_start(out=vq_d[0:1, :], in_=zrow_sb[:])

        # h rows (residual / layer-1 input), SBUF-resident
        had_sb = cpool.tile([P, NGROUP, CDIM], f32, tag="had")

        # ------------------------------------------------------------------
        def phase1(l):
            with tc.tile_pool(name=f"ps1_{l}", bufs=2, space="PSUM") as pp:
                for b0 in range(0, NGROUP, TB):
                    recb = p1.tile([P, TB, RECP], bf16, tag="recb")
                    vqb = p1.tile([P, TB, 8], bf16, tag="vqb")
                    for tt in range(TB):
                        t = b0 + tt
                        if l == 0:
                            ph = pp.tile([P, CDIM], f32, tag="ph")
                            nc.tensor.matmul(ph[:],
                                             lhsT=xTa_sb[:, t * P:(t + 1) * P],
                                             rhs=nodeWT_sb[:], start=True,
                                             stop=True)
                            drt = p1.tile([P, CDIM], f32, tag="drt")
                            nc.gpsimd.indirect_dma_start(
                                out=drt[:], out_offset=None, in_=dr64_d[:],
                                in_offset=bass.IndirectOffsetOnAxis(
                                    ap=batchc_sb[:, t:t + 1], axis=0))
                            nc.vector.tensor_add(had_sb[:, t, :], ph[:],
                                                 drt[:])
                        pt = pp.tile([CDIM, P], f32, tag="pt")
                        nc.tensor.transpose(pt[:], had_sb[:, t, :],
                                            ident_sb[:])
                        hT = p1.tile([CDIM, P], f32, tag="hT")
                        nc.scalar.copy(hT[:], pt[:])
                        prc = pp.tile([P, REC], f32, tag="prc")
                        nc.tensor.matmul(prc[:], lhsT=hT[:],
                                         rhs=wcomb_sb[l][:], start=True,
                                         stop=True)
                        nc.scalar.copy(recb[:, tt, 0:HC], prc[:, 0:HC])
                        nc.scalar.activation(recb[:, tt, HC:HC + H],
                                             prc[:, HC:HC + H], Act.Exp)
                        nc.scalar.activation(recb[:, tt, HC + H:REC],
                                             prc[:, HC:HC + H], Act.Exp,
                                             scale=0.2)
                        nc.scalar.activation(vqb[:, tt, 0:4],
                                             prc[:, HC + H:REC], Act.Exp)
                        nc.scalar.activation(vqb[:, tt, 4:8],
                                             prc[:, HC + H:REC], Act.Exp,
                                             scale=0.2)
                    r0 = b0 * P
                    rows = TB * P
                    nc.sync.dma_start(
                        out=rec_own_d[r0:r0 + rows, :].rearrange(
                            "(c p) f -> p c f", p=P),
                        in_=recb[:, :, :])
                    nc.sync.dma_start(
                        out=vq_d[1 + r0:1 + r0 + rows, 0:8].rearrange(
                            "(c p) f -> p c f", p=P),
                        in_=vqb[:, :, :])
            nc.gpsimd.collective_compute(
                "AllGather", mybir.AluOpType.bypass,
                replica_groups=[list(range(NCORES))],
                ins=[rec_own_d[:, :].opt()],
                outs=[rec_d[:, :].opt()])

        # ------------------------------------------------------------------
        def phase2(l):
            with tc.tile_pool(name=f"ps2_{l}", bufs=2, space="PSUM") as pp:
                col0 = 0
                for g in range(NGROUP):
                    CH = cfg.chg[g]
                    idxt = p2.tile([P, CHMAX * 8], i16, tag="idxt")
                    nc.sync.dma_start(out=idxt[:, :CH * 8],
                                      in_=idx128_d[:, col0 * 8:(col0 + CH) * 8])
                    ds8t = p2.tile([P, CHMAX * 8], i8, tag="ds8t")
                    nc.sync.dma_start(
                        out=ds8t[:, :CH * 8],
                        in_=dsw128_d[:, col0 * 8:(col0 + CH) * 8])
                    vqit = p2.tile([P, CHMAX * 8], i16, tag="vqit")
                    nc.vector.tensor_copy(vqit[:, :CH * 8], ds8t[:, :CH * 8])
                    nc.vector.tensor_scalar(vqit[:, :CH * 8],
                                            vqit[:, :CH * 8], g * P + 1,
                                            None, Alu.add)
                    rect = p2.tile([P, CHMAX, RECP], bf16, tag="rect")
                    done = 0
                    i = 0
                    while done < CH:   # HW envelope: <=256 idxs per call
                        st = min(2, CH - done)
                        base = cfg.callbase[g][i]
                        nrows = min(BUCKET, NPAD - base)
                        nc.gpsimd.dma_gather(
                            rect[:, done:done + st, :],
                            rec_d[base:base + nrows, :],
                            idxt[:, done * 8:(done + st) * 8],
                            st * P, st * P, RECP)
                        done += st
                        i += 1
                    vqt = p2.tile([P, CHMAX, VQW], bf16, tag="vqt")
                    done = 0
                    while done < CH:
                        st = min(2, CH - done)
                        nc.gpsimd.dma_gather(
                            vqt[:, done:done + st, :], vq_d[0:NPC + 1, :],
                            vqit[:, done * 8:(done + st) * 8],
                            st * P, st * P, VQW)
                        done += st
                    # one-hot M[edge, dst_slot]
                    Mt = p2.tile([P, CHMAX, P], bf16, tag="Mt")
                    nc.vector.tensor_tensor(
                        Mt[:, :CH, :],
                        ds8_sb[:, col0:col0 + CH][:, :, None].to_broadcast(
                            [P, CH, P]),
                        iota8_sb[:, None, :].to_broadcast([P, CH, P]),
                        Alu.is_equal)
                    # ex = max(u*v, p*q) -> rect[..., 256:260]
                    t1 = p2.tile([P, CHMAX, H], f32, tag="t1")
                    nc.vector.tensor_tensor(t1[:, :CH, :],
                                            rect[:, :CH, HC:HC + H],
                                            vqt[:, :CH, 0:4], Alu.mult)
                    t2 = p2.tile([P, CHMAX, H], f32, tag="t2")
                    nc.vector.tensor_tensor(t2[:, :CH, :],
                                            rect[:, :CH, HC + H:REC],
                                            vqt[:, :CH, 4:8], Alu.mult)
                    nc.vector.tensor_tensor(rect[:, :CH, HC:HC + H],
                                            t1[:, :CH, :], t2[:, :CH, :],
                                            Alu.max)
                    # V = ex * xh (per head, in place)
                    for h_ in range(H):
                        nc.vector.tensor_tensor(
                            rect[:, :CH, h_ * CDIM:(h_ + 1) * CDIM],
                            rect[:, :CH, h_ * CDIM:(h_ + 1) * CDIM],
                            rect[:, :CH, HC + h_:HC + h_ + 1].to_broadcast(
                                [P, CH, CDIM]),
                            Alu.mult)
                    # contract over edges: pg[:, 0:256]=sum ex*xh, [256:260]=s
                    pg = pp.tile([P, HC + H], f32, tag="pg")
                    for c in range(CH):
                        nc.tensor.matmul(pg[:], lhsT=Mt[:, c, :],
                                         rhs=rect[:, c, 0:HC + H],
                                         start=(c == 0), stop=(c == CH - 1))
                    # r = 1 / (s + eps) / H
                    s4 = p2.tile([P, H], f32, tag="s4")
                    nc.vector.tensor_scalar(s4[:], pg[:, HC:HC + H], 1e-16,
                                            None, Alu.add)
                    r4 = p2.tile([P, H], f32, tag="r4")
                    nc.vector.reciprocal(r4[:], s4[:])
                    nc.vector.tensor_scalar_mul(r4[:], r4[:], 1.0 / H)
                    # head mean
                    yt = p2.tile([P, CDIM], f32, tag="yt")
                    tmp = p2.tile([P, CDIM], f32, tag="tmp")
                    nc.vector.tensor_scalar(yt[:], pg[:, 0:CDIM], r4[:, 0:1],
                                            None, Alu.mult)
                    for h_ in range(1, H):
                        nc.vector.tensor_scalar(
                            tmp[:], pg[:, h_ * CDIM:(h_ + 1) * CDIM],
                            r4[:, h_:h_ + 1], None, Alu.mult)
                        nc.vector.tensor_add(yt[:], yt[:], tmp[:])
                    nc.vector.tensor_add(yt[:], yt[:], convb_sb[l][:])
                    # layernorm
                    mu = p2.tile([P, 1], f32, tag="mu")
                    nc.vector.tensor_reduce(mu[:], yt[:], mybir.AxisListType.X,
                                            Alu.add)
                    nc.vector.tensor_scalar_mul(mu[:], mu[:], 1.0 / CDIM)
                    nc.vector.tensor_scalar(yt[:], yt[:], mu[:, 0:1], None,
                                            Alu.subtract)
                    sq = p2.tile([P, CDIM], f32, tag="sq")
                    var = p2.tile([P, 1], f32, tag="var")
                    nc.scalar.activation(sq[:], yt[:], Act.Square,
                                         accum_out=var[:])
                    nc.vector.tensor_scalar(var[:], var[:], 1.0 / CDIM,
                                            LN_EPS, Alu.mult, Alu.add)
                    sd = p2.tile([P, 1], f32, tag="sd")
                    nc.scalar.sqrt(sd[:], var[:])
                    inv = p2.tile([P, 1], f32, tag="inv")
                    nc.vector.reciprocal(inv[:], sd[:])
                    nc.vector.tensor_scalar(yt[:], yt[:], inv[:, 0:1], None,
                                            Alu.mult)
                    nc.vector.tensor_mul(yt[:], yt[:], lng_sb[l][:])
                    nc.vector.tensor_add(yt[:], yt[:], lnb_sb[l][:])
                    nc.vector.tensor_scalar_max(yt[:], yt[:], 0.0)
                    # residual
                    if l == 0:
                        nc.vector.tensor_add(had_sb[:, g, :], yt[:],
                                             had_sb[:, g, :])
                    else:
                        yt2 = p2.tile([P, CDIM], f32, tag="yt2")
                        nc.vector.tensor_add(yt2[:], yt[:], had_sb[:, g, :])
                        pt2 = pp.tile([CDIM, P], f32, tag="pt2")
                        nc.tensor.transpose(pt2[:], yt2[:], ident_sb[:])
                        hT2 = p2.tile([CDIM, P], f32, tag="hT2")
                        nc.scalar.copy(hT2[:], pt2[:])
                        po = pp.tile([P, OUT_F], f32, tag="po")
                        nc.tensor.matmul(po[:], lhsT=hT2[:], rhs=outWT_sb[:],
                                         start=True, stop=True)
                        ot = p2.tile([P, OUT_F], f16, tag="ot")
                        nc.vector.tensor_add(ot[:], po[:], outb_sb[:])
                        nc.sync.dma_start(out=out_d[g * P:(g + 1) * P, :],
                                          in_=ot[:])
                    col0 += CH

        # ------------------------------------------------------------------
        phase1(0)
        phase2(0)
        phase1(1)
        phase2(1)

    nc.compile()
    return nc


# --------------------------------------------------------------------------
# entry point
# --------------------------------------------------------------------------

def _in_maps(cfg, prep, shared, per_core_w):
    lay, blob_bytes = _blob_layout(cfg.cols)
    maps = []
    for k in range(NCORES):
        src = dict(shared)
        src.update(prep["per_core"][k])
        src.update(per_core_w[k])
        blob = np.zeros((1, blob_bytes), np.int8)
        for nm, (off, sh, dt, nb) in lay.items():
            a = np.ascontiguousarray(np.asarray(src[nm], dt))
            assert list(a.shape) == list(sh), (nm, a.shape, sh)
            blob[0, off:off + nb] = a.view(np.int8).ravel()
        maps.append(dict(blob=blob))
    return maps


def _gather_out(prep, results):
    out = np.empty((N, OUT_F), np.float32)
    order = prep["order"]
    for k in range(NCORES):
        blk = order[k * NPC:(k + 1) * NPC]
        valid = blk >= 0
        out[blk[valid]] = results[k]["out"][valid].astype(np.float32)
    return out


def kernel(**inputs):
    edge_index = np.asarray(inputs["edge_index"])
    prep = _host_prep(edge_index)
    cfg = prep["cfg"]
    shared, per_core_w = _host_weights(inputs, prep["order"])
    nc = _build(cfg)
    maps = _in_maps(cfg, prep, shared, per_core_w)

    from concourse import bass_utils
    res = bass_utils.run_bass_kernel_spmd(nc, maps,
                                          core_ids=list(range(NCORES)))
    return _gather_out(prep, res.results)
